# revision 16
# baseline (speedup 1.0000x reference)
"""Trainium2 Bass kernel for nn_BasicConvolutionBlock (sparse conv + BN + ReLU).

Math (per reference):
    conv[n] = sum_k feats[nbr_idx[n, k]] @ W[k]       # [N, 96], k = 0..26
    y = (conv - mean) * rsqrt(var + eps) * gamma + beta ; relu(y)

Distribution: voxel (N) dimension sharded across 8 NeuronCores; feats table
(bf16, channel-padded to 128) and weights replicated per core.

Gather strategy (the hot loop): dma_gather (InstDMAGatherAnt) moves ~16
random 256B rows per DMA descriptor, but takes int16 indices (< 32768).
The feats table has 262144 rows, so each 512-voxel tile does a two-level
gather:
  L1: the tile's 13824 (offset, voxel) row-indices are bucketed by table
      chunk (8 chunks x 32768 rows) on the host; one dma_gather per chunk
      (2048-slot budget, dummy index 0 padding) pulls the rows chunk-local
      -> SBUF [128, 128, 128ch] bf16, then staged to a DRAM scratch region
      (16384 rows).
  L2: one dma_gather from the scratch (indices < 16384) restores
      (k, block, partition) order -> [128, 108, 128ch] bf16.
Per offset k: 4 PE transposes -> PSUM [128, 512], DVE copy -> bf16 rhs,
accumulating bf16 matmul W_k.T @ rhs -> PSUM [96, 512].
BN partial sum/sumsq via ACT accum; conv staged to DRAM channel-major.

Two NEFFs (the AllReduce-in-kernel path is unstable under the axon PJRT
bridge, so per-core BN partial sums are combined on the host -- 768 B of
float math -- between the two device passes):
  pass 2: y = relu(conv * scale + shift) (fused ACT op), PE transpose back
    to row-major, store. The host index permutation is chosen so pass2's
    output DMA writes contiguous 6KB runs per partition.
"""
import numpy as np
import ml_dtypes

import concourse.bass as bass
import concourse.bacc as bacc
import concourse.tile as tile
import concourse.mybir as mybir
from concourse.masks import make_identity

F32 = mybir.dt.float32
BF16 = mybir.dt.bfloat16
I16 = mybir.dt.int16
AF = mybir.ActivationFunctionType

N_TOTAL = 262144
C = 96
CP = 128                             # channel-padded row (256B bf16)
KVOL = 27
N_CORES = 8
N_PER_CORE = N_TOTAL // N_CORES      # 32768
TILE_V = 512                         # voxels per tile
BLOCKS = TILE_V // 128               # 4
J = KVOL * BLOCKS                    # 108 gathered row-blocks per tile
N_TILES = N_PER_CORE // TILE_V       # 64
NCHUNK = 8
CHUNK_ROWS = N_TOTAL // NCHUNK       # 32768 (int16-addressable)
SLOT_BUDGET = 2048                   # slots per (tile, chunk), mult of 128
SLOTS = NCHUNK * SLOT_BUDGET         # 16384 per tile
BN_EPS = 1e-5

_cache = {}


# --------------------------------------------------------------------------
# graph builders
# --------------------------------------------------------------------------
NI_MAX = 1024                        # hw limit: indices per dma_gather


def pass1_body(nc, feats_bf, idx1, idx2, Wt, convT, stats, n_tiles,
               tc=None, cpool=None):
    """Sparse-conv pass: gathers + matmuls + BN partial sums.

    When tc/cpool are given (fused single-NEFF build), runs inside the
    caller's TileContext and leaves stats in cpool tiles; `stats` may then
    be None. Standalone, creates its own context and writes `stats`."""
    scratch = nc.dram_tensor("scratch", [n_tiles, SLOTS, CP], BF16)
    nstripe = NI_MAX // 128          # stripes written per L1 gather
    l1_per_chunk = SLOT_BUDGET // NI_MAX

    import contextlib
    own_ctx = tc is None
    ctx = contextlib.ExitStack()
    with ctx:
        if own_ctx:
            tc = ctx.enter_context(tile.TileContext(nc))
            cpool = ctx.enter_context(tc.tile_pool(name="const", bufs=1))
        with (
            tc.tile_pool(name="gp", bufs=2) as gpool,
            tc.tile_pool(name="g2p", bufs=2) as g2pool,
            tc.tile_pool(name="rp", bufs=3) as rpool,
            tc.tile_pool(name="sp", bufs=2) as spool,
            tc.tile_pool(name="psA", bufs=2, space="PSUM") as psA,
            tc.tile_pool(name="psB", bufs=2, space="PSUM") as psB,
        ):
            ident = cpool.tile([128, 128], BF16, tag="ident")
            make_identity(nc, ident[:])
            w_sb = cpool.tile([CP, KVOL, C], BF16, tag="w")
            nc.sync.dma_start(w_sb[:], Wt[:])

            sum_acc = cpool.tile([C, n_tiles], F32, tag="sum_acc")
            sq_acc = cpool.tile([C, n_tiles], F32, tag="sq_acc")

            for t in range(n_tiles):
                i1 = gpool.tile([128, NCHUNK, SLOT_BUDGET // 16], I16,
                                tag="i1")
                nc.sync.dma_start(i1[:], idx1[t])
                i2 = gpool.tile([128, J * 128 // 16], I16, tag="i2")
                nc.sync.dma_start(i2[:], idx2[t])

                gs = gpool.tile([128, SLOTS // 128, CP], BF16, tag="gs")
                for c in range(NCHUNK):
                    for h in range(l1_per_chunk):
                        s0 = (c * l1_per_chunk + h) * nstripe
                        nc.gpsimd.dma_gather(
                            out_ap=gs[:, s0:s0 + nstripe, :],
                            in_ap=feats_bf[c * CHUNK_ROWS:
                                           (c + 1) * CHUNK_ROWS, :],
                            idxs_ap=i1[:, c, h * (NI_MAX // 16):
                                       (h + 1) * (NI_MAX // 16)],
                            num_idxs=NI_MAX,
                            num_idxs_reg=NI_MAX,
                            elem_size=CP,
                        )
                # stage to DRAM: slot s -> scratch row (s%128)*128 + s//128
                nc.sync.dma_start(
                    scratch[t].rearrange("(p s) c -> p s c", p=128), gs[:])
                # gather back in (k, block, partition) order
                g2 = g2pool.tile([128, J, CP], BF16, tag="g2")
                for m in range((J * 128 + NI_MAX - 1) // NI_MAX):
                    ni = min(NI_MAX, J * 128 - m * NI_MAX)
                    nc.gpsimd.dma_gather(
                        out_ap=g2[:, m * nstripe:m * nstripe + ni // 128, :],
                        in_ap=scratch[t],
                        idxs_ap=i2[:, m * (NI_MAX // 16):
                                   m * (NI_MAX // 16) + ni // 16],
                        num_idxs=ni,
                        num_idxs_reg=ni,
                        elem_size=CP,
                    )

                out_ps = psA.tile([C, TILE_V], F32, tag="outp")
                for k in range(KVOL):
                    tp = psB.tile([128, TILE_V], BF16, tag="tp")
                    for b in range(BLOCKS):
                        nc.tensor.transpose(
                            tp[:, b * 128:(b + 1) * 128],
                            g2[:, k * BLOCKS + b, :],
                            ident[:],
                        )
                    rhs = rpool.tile([128, TILE_V], BF16, tag="rhs")
                    nc.vector.tensor_copy(rhs[:], tp[:])
                    nc.tensor.matmul(
                        out_ps[:], w_sb[:, k, :], rhs[:],
                        start=(k == 0), stop=(k == KVOL - 1),
                    )

                conv_sb = spool.tile([C, TILE_V], F32, tag="conv")
                nc.scalar.activation(
                    conv_sb[:], out_ps[:], AF.Identity,
                    accum_out=sum_acc[:, t:t + 1])
                sq_sb = spool.tile([C, TILE_V], F32, tag="sq")
                nc.scalar.activation(
                    sq_sb[:], conv_sb[:], AF.Square,
                    accum_out=sq_acc[:, t:t + 1])
                nc.sync.dma_start(
                    convT[:, t * TILE_V:(t + 1) * TILE_V], conv_sb[:])

            stats_sb = cpool.tile([C, 2], F32, tag="stats_sb")
            nc.vector.reduce_sum(
                stats_sb[:, 0:1], sum_acc[:], axis=mybir.AxisListType.X)
            nc.vector.reduce_sum(
                stats_sb[:, 1:2], sq_acc[:], axis=mybir.AxisListType.X)
            if stats is not None:
                nc.sync.dma_start(stats[:], stats_sb[:])
    return stats_sb


def pass2_body(nc, tc, cpool, convT, scale_sb, shift_sb, out, n_tiles):
    """Normalize + ReLU + transpose back to row-major."""
    tile_v2, n_tiles2 = _tile2(n_tiles)
    blocks2 = tile_v2 // 128
    with (
        tc.tile_pool(name="sp2", bufs=3) as spool,
        tc.tile_pool(name="ps2", bufs=4, space="PSUM") as ps,
    ):
        ident = cpool.tile([C, C], F32, tag="ident2")
        make_identity(nc, ident[:])
        for t in range(n_tiles2):
            nsb = spool.tile([C, tile_v2], F32, tag="nsb")
            nc.sync.dma_start(
                nsb[:], convT[:, t * tile_v2:(t + 1) * tile_v2])
            nrm = spool.tile([C, tile_v2], F32, tag="nrm")
            nc.scalar.activation(
                nrm[:], nsb[:], AF.Relu,
                bias=shift_sb[:], scale=scale_sb[:])
            # conv column b*128 + p holds voxel t*tile_v2 + p*blocks2 + b
            # (host vperm), so partition p's osb row is a contiguous run of
            # blocks2 output rows.
            osb = spool.tile([128, blocks2 * C], F32, tag="osb")
            for g in range(0, blocks2, 4):
                gw = min(4, blocks2 - g)
                op = ps.tile([128, 4 * C], F32, tag="op")
                for bi in range(gw):
                    nc.tensor.transpose(
                        op[:, bi * C:(bi + 1) * C],
                        nrm[:, (g + bi) * 128:(g + bi + 1) * 128],
                        ident[:],
                    )
                nc.vector.tensor_copy(
                    osb[:, g * C:(g + gw) * C], op[:, :gw * C])
            nc.sync.dma_start(
                out[t * tile_v2:(t + 1) * tile_v2, :].rearrange(
                    "(p b) c -> p b c", p=128),
                osb[:].rearrange("p (b c) -> p b c", b=blocks2),
            )


def build_fused(n_tiles=N_TILES, n_cores=N_CORES):
    """Single NEFF: conv + BN-stats AllReduce + normalize/ReLU."""
    nc = bacc.Bacc("TRN2", target_bir_lowering=False, debug=False,
                   num_devices=n_cores)
    feats_bf = nc.dram_tensor("feats_bf", [N_TOTAL, CP], BF16,
                              kind="ExternalInput")
    idx1 = nc.dram_tensor("idx1", [n_tiles, 128, NCHUNK, SLOT_BUDGET // 16],
                          I16, kind="ExternalInput")
    idx2 = nc.dram_tensor("idx2", [n_tiles, 128, J * 128 // 16], I16,
                          kind="ExternalInput")
    Wt = nc.dram_tensor("Wt", [CP, KVOL, C], BF16, kind="ExternalInput")
    gamma = nc.dram_tensor("gamma", [C, 1], F32, kind="ExternalInput")
    beta = nc.dram_tensor("beta", [C, 1], F32, kind="ExternalInput")
    out = nc.dram_tensor("out", [n_tiles * TILE_V, C], F32,
                         kind="ExternalOutput")
    convT = nc.dram_tensor("convT", [C, n_tiles * TILE_V], F32)
    bnsrc = nc.dram_tensor("bnsrc", [C, 2], F32)
    bnred = nc.dram_tensor("bnred", [C, 2], F32, addr_space="Shared")
    n_total = n_tiles * TILE_V * n_cores

    with tile.TileContext(nc) as tc:
        with tc.tile_pool(name="const", bufs=1) as cpool:
            stats_sb = pass1_body(nc, feats_bf, idx1, idx2, Wt, convT, None,
                                  n_tiles, tc=tc, cpool=cpool)
            # global BN stats: AllReduce the [C, 2] partial sums
            nc.sync.dma_start(bnsrc[:], stats_sb[:])
            nc.gpsimd.collective_compute(
                "AllReduce", mybir.AluOpType.add,
                replica_groups=[list(range(n_cores))],
                ins=[bnsrc[:]], outs=[bnred[:]],
            )
            st2 = cpool.tile([C, 2], F32, tag="st2")
            nc.sync.dma_start(st2[:], bnred[:])
            gam = cpool.tile([C, 1], F32, tag="gam")
            bet = cpool.tile([C, 1], F32, tag="bet")
            nc.sync.dma_start(gam[:], gamma[:])
            nc.sync.dma_start(bet[:], beta[:])

            # scale = gamma * rsqrt(var + eps); shift = beta - mean * scale
            mm = cpool.tile([C, 2], F32, tag="mm")
            nc.vector.tensor_scalar(
                mm[:], st2[:], 1.0 / n_total, None, mybir.AluOpType.mult)
            msq = cpool.tile([C, 1], F32, tag="msq")
            nc.vector.tensor_tensor(
                msq[:], mm[:, 0:1], mm[:, 0:1], mybir.AluOpType.mult)
            var = cpool.tile([C, 1], F32, tag="var")
            nc.vector.tensor_tensor(
                var[:], mm[:, 1:2], msq[:], mybir.AluOpType.subtract)
            epst = cpool.tile([C, 1], F32, tag="eps")
            nc.vector.memset(epst[:], BN_EPS)
            std = cpool.tile([C, 1], F32, tag="std")
            nc.scalar.activation(std[:], var[:], AF.Sqrt, bias=epst[:])
            inv = cpool.tile([C, 1], F32, tag="inv")
            nc.vector.reciprocal(inv[:], std[:])
            scale_sb = cpool.tile([C, 1], F32, tag="scale")
            nc.vector.tensor_tensor(
                scale_sb[:], gam[:], inv[:], mybir.AluOpType.mult)
            mscale = cpool.tile([C, 1], F32, tag="mscale")
            nc.vector.tensor_tensor(
                mscale[:], mm[:, 0:1], scale_sb[:], mybir.AluOpType.mult)
            shift_sb = cpool.tile([C, 1], F32, tag="shift")
            nc.vector.tensor_tensor(
                shift_sb[:], bet[:], mscale[:], mybir.AluOpType.subtract)

            pass2_body(nc, tc, cpool, convT, scale_sb, shift_sb, out, n_tiles)

    nc.finalize()
    return nc


def build_pass1(n_tiles=N_TILES, n_cores=N_CORES):
    nc = bacc.Bacc("TRN2", target_bir_lowering=False, debug=False,
                   num_devices=n_cores)
    feats_bf = nc.dram_tensor("feats_bf", [N_TOTAL, CP], BF16,
                              kind="ExternalInput")
    idx1 = nc.dram_tensor("idx1", [n_tiles, 128, NCHUNK, SLOT_BUDGET // 16],
                          I16, kind="ExternalInput")
    idx2 = nc.dram_tensor("idx2", [n_tiles, 128, J * 128 // 16], I16,
                          kind="ExternalInput")
    Wt = nc.dram_tensor("Wt", [CP, KVOL, C], BF16, kind="ExternalInput")
    convT = nc.dram_tensor("convT", [C, n_tiles * TILE_V], F32,
                           kind="ExternalOutput")
    stats = nc.dram_tensor("stats", [C, 2], F32, kind="ExternalOutput")
    pass1_body(nc, feats_bf, idx1, idx2, Wt, convT, stats, n_tiles)
    nc.finalize()
    return nc


def _tile2(n_tiles):
    """Pass-2 tiling: [tile_v2, n_tiles2]."""
    total = n_tiles * TILE_V
    tile_v2 = min(2048, total)
    return tile_v2, total // tile_v2


def build_pass2(n_tiles=N_TILES):
    nc = bacc.Bacc("TRN2", target_bir_lowering=False, debug=False,
                   num_devices=N_CORES)
    tile_v2, n_tiles2 = _tile2(n_tiles)
    blocks2 = tile_v2 // 128
    convT = nc.dram_tensor("convT", [C, n_tiles * TILE_V], F32,
                           kind="ExternalInput")
    scale = nc.dram_tensor("scale", [C, 1], F32, kind="ExternalInput")
    shift = nc.dram_tensor("shift", [C, 1], F32, kind="ExternalInput")
    out = nc.dram_tensor("out", [n_tiles * TILE_V, C], F32,
                         kind="ExternalOutput")

    with tile.TileContext(nc) as tc:
        with (
            tc.tile_pool(name="const", bufs=1) as cpool,
            tc.tile_pool(name="sp", bufs=3) as spool,
            tc.tile_pool(name="ps", bufs=4, space="PSUM") as ps,
        ):
            ident = cpool.tile([C, C], F32, tag="ident")
            make_identity(nc, ident[:])
            scale_sb = cpool.tile([C, 1], F32, tag="scale")
            shift_sb = cpool.tile([C, 1], F32, tag="shift")
            nc.sync.dma_start(scale_sb[:], scale[:])
            nc.sync.dma_start(shift_sb[:], shift[:])

            for t in range(n_tiles2):
                nsb = spool.tile([C, tile_v2], F32, tag="nsb")
                nc.sync.dma_start(
                    nsb[:], convT[:, t * tile_v2:(t + 1) * tile_v2])
                nrm = spool.tile([C, tile_v2], F32, tag="nrm")
                nc.scalar.activation(
                    nrm[:], nsb[:], AF.Relu,
                    bias=shift_sb[:], scale=scale_sb[:])
                # conv column b*128 + p holds voxel t*tile_v2 + p*blocks2 + b
                # (host vperm), so partition p's osb row is a contiguous run
                # of blocks2 output rows.
                osb = spool.tile([128, blocks2 * C], F32, tag="osb")
                for g in range(0, blocks2, 4):
                    gw = min(4, blocks2 - g)
                    op = ps.tile([128, 4 * C], F32, tag="op")
                    for bi in range(gw):
                        nc.tensor.transpose(
                            op[:, bi * C:(bi + 1) * C],
                            nrm[:, (g + bi) * 128:(g + bi + 1) * 128],
                            ident[:],
                        )
                    nc.vector.tensor_copy(
                        osb[:, g * C:(g + gw) * C], op[:, :gw * C])
                nc.sync.dma_start(
                    out[t * tile_v2:(t + 1) * tile_v2, :].rearrange(
                        "(p b) c -> p b c", p=128),
                    osb[:].rearrange("p (b c) -> p b c", b=blocks2),
                )

    nc.finalize()
    return nc


# --------------------------------------------------------------------------
# reusable PJRT runner (keeps the compiled executable across calls)
# --------------------------------------------------------------------------
class _Runner:
    """Runs a bass NEFF over n_cores devices via shard_map.

    `replicated`: input names fed once (same array on every core).
    Inputs/outputs are jax arrays; sharded inputs are globally concatenated
    on axis 0 (core-major). Outputs stay on device until converted.
    """

    def __init__(self, nc, n_cores, replicated=()):
        import jax
        from jax.sharding import Mesh, PartitionSpec
        from jax.experimental.shard_map import shard_map
        from concourse import bass2jax

        bass2jax.install_neuronx_cc_hook()
        self.jax = jax
        self.n_cores = n_cores
        self.replicated = set(replicated)
        pname = nc.partition_id_tensor.name if nc.partition_id_tensor else None
        in_names, out_names, out_avals, zero_outs = [], [], [], []
        for alloc in nc.m.functions[0].allocations:
            if not isinstance(alloc, mybir.MemoryLocationSet):
                continue
            name = alloc.memorylocations[0].name
            if alloc.kind == "ExternalInput":
                if name != pname:
                    in_names.append(name)
            elif alloc.kind == "ExternalOutput":
                out_names.append(name)
                shape = tuple(alloc.tensor_shape)
                dtype = mybir.dt.np(alloc.dtype)
                out_avals.append(jax.core.ShapedArray(shape, dtype))
                zero_outs.append(np.zeros(shape, dtype))
        self.in_names, self.out_names = in_names, out_names
        self.out_avals, self.zero_outs = out_avals, zero_outs
        n_params = len(in_names)
        self.n_params = n_params
        all_in = list(in_names) + list(out_names)
        if pname is not None:
            all_in.append(pname)

        def _body(*args):
            operands = list(args)
            if pname is not None:
                operands.append(bass2jax.partition_id_tensor())
            outs = bass2jax._bass_exec_p.bind(
                *operands,
                out_avals=tuple(out_avals),
                in_names=tuple(all_in),
                out_names=tuple(out_names),
                lowering_input_output_aliases=(),
                sim_require_finite=True,
                sim_require_nnan=True,
                nc=nc,
            )
            return tuple(outs)

        devices = jax.devices()[:n_cores]
        self.mesh = Mesh(np.asarray(devices), ("core",))
        self.in_specs = tuple(
            PartitionSpec() if n in self.replicated else PartitionSpec("core")
            for n in in_names
        ) + (PartitionSpec("core"),) * len(out_names)
        self.fn = jax.jit(
            shard_map(_body, mesh=self.mesh, in_specs=self.in_specs,
                      out_specs=(PartitionSpec("core"),) * len(out_names),
                      check_rep=False),
            keep_unused=True,
        )

    def prep(self, in_map):
        """in_map: replicated name -> array; sharded name -> list of per-core
        arrays OR pre-concatenated global array / jax array."""
        args = []
        for n in self.in_names:
            v = in_map[n]
            if isinstance(v, list):
                v = np.concatenate([np.asarray(x) for x in v], axis=0)
            args.append(v)
        args += [
            np.zeros((self.n_cores * z.shape[0], *z.shape[1:]), z.dtype)
            for z in self.zero_outs
        ]
        return args

    def prep_sharded(self, in_map):
        """Like prep, but device_put each arg with its target sharding so a
        subsequent fn(*args) does no data movement."""
        from jax.sharding import NamedSharding
        args = self.prep(in_map)
        out = []
        for a, spec in zip(args, self.in_specs):
            if not isinstance(a, self.jax.Array):
                a = self.jax.device_put(a, NamedSharding(self.mesh, spec))
            out.append(a)
        self.jax.block_until_ready(out)
        return out

    def run(self, in_map):
        outs = self.fn(*self.prep(in_map))
        self.jax.block_until_ready(outs)
        return dict(zip(self.out_names, outs))

    def percore(self, arr_global, name):
        i = self.out_names.index(name)
        return np.asarray(arr_global).reshape(
            self.n_cores, *self.out_avals[i].shape)


# --------------------------------------------------------------------------
# host-side glue
# --------------------------------------------------------------------------
def _vox_map(n_tiles):
    """vox[t, v]: shard-local voxel id gathered into conv column (t, v).

    Chosen so pass2's per-partition output rows are contiguous:
    for t = t2*G + q, v = b1*128 + p:
        vox = t2*tile_v2 + p*(4G) + q*4 + b1
    """
    tile_v2, n_t2 = _tile2(n_tiles)
    G = tile_v2 // TILE_V
    t2 = np.arange(n_t2)[:, None, None, None]
    q = np.arange(G)[None, :, None, None]
    b1 = np.arange(4)[None, None, :, None]
    p = np.arange(128)[None, None, None, :]
    vox = t2 * tile_v2 + p * (4 * G) + q * 4 + b1    # [n_t2, G, 4, 128]
    return vox.reshape(n_tiles, TILE_V)


def _wrap16(a):
    """[..., n] -> [..., 128, n//16]: position i -> (partition i%16, col
    i//16), replicated across the 8 groups of 16 partitions."""
    *lead, n = a.shape
    b = a.reshape(*lead, n // 16, 16)
    b = np.moveaxis(b, -1, -2)                       # [..., 16, n//16]
    return np.ascontiguousarray(
        np.broadcast_to(b[..., None, :, :],
                        (*lead, 8, 16, n // 16)).reshape(*lead, 128, n // 16))


def _arrange_idx(nbr_shard: np.ndarray, n_tiles: int):
    """Build (idx1, idx2) int16 arrays for one core shard.

    idx1 [n_tiles, 128, NCHUNK, SLOT_BUDGET//16]: L1 chunk-local row indices
        (slot-position order, dummy 0 padding).
    idx2 [n_tiles, 128, J*128//16]: L2 scratch-row indices restoring
        (k, block, partition) order.
    """
    vox = _vox_map(n_tiles)                          # [n_tiles, 512]
    R = nbr_shard[vox]                               # [n_tiles, 512, 27]
    R = np.ascontiguousarray(R.transpose(0, 2, 1)).reshape(
        n_tiles, KVOL * TILE_V)                      # i = k*512 + v
    chunk = (R >> 15).astype(np.int64)               # [n_tiles, 13824]
    local = (R & 32767).astype(np.int16)

    n = KVOL * TILE_V
    counts = np.zeros((n_tiles, NCHUNK), np.int64)
    trow = np.repeat(np.arange(n_tiles), n)
    np.add.at(counts, (trow, chunk.reshape(-1)), 1)
    if counts.max() > SLOT_BUDGET:
        raise RuntimeError(f"chunk bucket overflow: {counts.max()}")
    starts = np.concatenate(
        [np.zeros((n_tiles, 1), np.int64), np.cumsum(counts, 1)[:, :-1]], 1)

    order = np.argsort(chunk, axis=1, kind="stable")
    sorted_chunk = np.take_along_axis(chunk, order, 1)
    within = np.arange(n)[None, :] - np.take_along_axis(
        starts, sorted_chunk, 1)
    rank = np.empty_like(within)
    np.put_along_axis(rank, order, within, 1)
    slot = chunk * SLOT_BUDGET + rank                # [n_tiles, 13824]

    l1 = np.zeros((n_tiles, SLOTS), np.int16)
    np.put_along_axis(l1, slot, local, 1)
    idx1 = _wrap16(l1.reshape(n_tiles, NCHUNK, SLOT_BUDGET))
    idx1 = np.ascontiguousarray(idx1.transpose(0, 2, 1, 3))

    dramrow = ((slot & 127) << 7) | (slot >> 7)      # (s%128)*128 + s//128
    idx2 = _wrap16(dramrow.astype(np.int16))
    return idx1, idx2


def _prep_pass1_inputs(feats, nbr, W, n_tiles):
    npc = n_tiles * TILE_V
    feats_bf = np.zeros((N_TOTAL, CP), ml_dtypes.bfloat16)
    feats_bf[:, :C] = feats.astype(ml_dtypes.bfloat16)
    Wt = np.zeros((CP, KVOL, C), ml_dtypes.bfloat16)
    Wt[:C] = W.transpose(1, 0, 2).astype(ml_dtypes.bfloat16)
    i1_all, i2_all = [], []
    for c in range(N_CORES):
        i1, i2 = _arrange_idx(nbr[c * npc:(c + 1) * npc], n_tiles)
        i1_all.append(i1)
        i2_all.append(i2)
    return {
        "feats_bf": feats_bf,
        "Wt": Wt,
        "idx1": np.concatenate(i1_all, 0),
        "idx2": np.concatenate(i2_all, 0),
    }


def _fused_in_map(feats, nbr, W, gamma, beta, n_tiles):
    in_map = _prep_pass1_inputs(feats, nbr, W, n_tiles)
    in_map["gamma"] = np.asarray(gamma, np.float32).reshape(C, 1)
    in_map["beta"] = np.asarray(beta, np.float32).reshape(C, 1)
    return in_map


def _fused_runner(n_tiles):
    key = ("fused", n_tiles)
    if key not in _cache:
        _cache[key] = _Runner(
            build_fused(n_tiles), N_CORES,
            replicated=("feats_bf", "Wt", "gamma", "beta"))
    return _cache[key]


def _two_pass_runners(n_tiles):
    key = ("p1", n_tiles)
    if key not in _cache:
        _cache[key] = _Runner(build_pass1(n_tiles), N_CORES,
                              replicated=("feats_bf", "Wt"))
    key2 = ("p2", n_tiles)
    if key2 not in _cache:
        _cache[key2] = _Runner(build_pass2(n_tiles), N_CORES,
                               replicated=("scale", "shift"))
    return _cache[key], _cache[key2]


def _host_bn(stats_percore, gamma, beta, n_total):
    """Combine per-core BN partial sums (768 B) into scale/shift."""
    s = stats_percore.sum(axis=0, dtype=np.float64)    # [96, 2]
    mean = s[:, 0] / n_total
    var = s[:, 1] / n_total - mean * mean
    inv = 1.0 / np.sqrt(var + BN_EPS)
    g = np.asarray(gamma, np.float64).reshape(C)
    b = np.asarray(beta, np.float64).reshape(C)
    scale = (g * inv).astype(np.float32)
    shift = (b - mean * g * inv).astype(np.float32)
    return scale.reshape(C, 1), shift.reshape(C, 1)


def run_pipeline(feats, nbr, W, gamma, beta, n_tiles):
    """Two NEFFs; the in-kernel AllReduce path (build_fused) measures ~6 ms
    slower with high variance under the axon bridge (software-emulated
    collectives), so BN stats are combined on the host instead."""
    r1, r2 = _two_pass_runners(n_tiles)
    res1 = r1.run(_prep_pass1_inputs(feats, nbr, W, n_tiles))
    stats = r1.percore(res1["stats"], "stats")         # [8, 96, 2]
    scale, shift = _host_bn(stats, gamma, beta, n_tiles * TILE_V * N_CORES)
    res2 = r2.run({"convT": res1["convT"], "scale": scale, "shift": shift})
    return np.asarray(res2["out"])


def kernel(feats, nbr_idx, W, gamma, beta):
    feats = np.ascontiguousarray(feats, dtype=np.float32)
    W = np.ascontiguousarray(W, dtype=np.float32)
    nbr = np.asarray(nbr_idx)
    gamma = np.asarray(gamma, dtype=np.float32)
    beta = np.asarray(beta, dtype=np.float32)
    return run_pipeline(feats, nbr, W, gamma, beta, N_TILES)


def _floor_runner():
    """Trivial 8-core kernel: measures the axon dispatch floor."""
    if "floor8" not in _cache:
        nc = bacc.Bacc("TRN2", target_bir_lowering=False, debug=False,
                       num_devices=N_CORES)
        x = nc.dram_tensor("x", [128, 128], F32, kind="ExternalInput")
        y = nc.dram_tensor("y", [128, 128], F32, kind="ExternalOutput")
        with tile.TileContext(nc) as tc:
            with tc.tile_pool(name="s", bufs=1) as p:
                t = p.tile([128, 128], F32, tag="t")
                nc.sync.dma_start(t[:], x[:])
                nc.vector.tensor_copy(t[:], t[:])
                nc.sync.dma_start(y[:], t[:])
        nc.finalize()
        _cache["floor8"] = _Runner(nc, N_CORES)
    return _cache["floor8"]


def _paired_diff(rf, args_f, rk, args_k, iters):
    """Median of adjacent (kernel - floor) wall-time differences. The axon
    dispatch overhead (~60-110 ms) drifts on a seconds scale; adjacent
    pairing cancels it. The 8 per-core NEFFs execute in parallel, so the
    difference is the per-core device time."""
    import time
    import jax

    jax.block_until_ready(rf.fn(*args_f))
    jax.block_until_ready(rk.fn(*args_k))
    diffs, floors, kerns = [], [], []
    for _ in range(iters):
        t0 = time.perf_counter()
        jax.block_until_ready(rf.fn(*args_f))
        tf = time.perf_counter() - t0
        t0 = time.perf_counter()
        jax.block_until_ready(rk.fn(*args_k))
        tk = time.perf_counter() - t0
        floors.append(tf)
        kerns.append(tk)
        diffs.append(tk - tf)
    return (float(np.median(diffs)), float(np.median(floors)),
            float(np.median(kerns)))


def measure_exec(feats, nbr_idx, W, gamma, beta, n_tiles=N_TILES, iters=14):
    """Paired-difference timing of both passes. Returns
    (pass1_s, pass2_s, floor_s)."""
    r1, r2 = _two_pass_runners(n_tiles)
    rf = _floor_runner()
    in1 = _prep_pass1_inputs(
        np.ascontiguousarray(feats, np.float32), np.asarray(nbr_idx),
        np.ascontiguousarray(W, np.float32), n_tiles)
    args1 = r1.prep_sharded(in1)
    args_f = rf.prep_sharded({"x": np.ones((128, 128), np.float32)})
    d1, f1, _ = _paired_diff(rf, args_f, r1, args1, iters)

    res1 = dict(zip(r1.out_names, r1.fn(*args1)))
    stats = r1.percore(np.asarray(res1["stats"]), "stats")
    scale, shift = _host_bn(stats, gamma, beta, n_tiles * TILE_V * N_CORES)
    args2 = r2.prep_sharded({
        "convT": res1["convT"], "scale": scale, "shift": shift})
    d2, f2, _ = _paired_diff(rf, args_f, r2, args2, iters)
    return d1, d2, (f1 + f2) / 2


# revision 20
# speedup vs baseline: 2425.9133x; 2425.9133x over previous
"""Trainium2 Bass kernel for nn_BasicConvolutionBlock (sparse conv + BN + ReLU).

Math (per reference):
    conv[n] = sum_k feats[nbr_idx[n, k]] @ W[k]       # [N, 96], k = 0..26
    y = (conv - mean) * rsqrt(var + eps) * gamma + beta ; relu(y)

Distribution: voxel (N) dimension sharded across 8 NeuronCores; feats table
(bf16, channel-padded to 128) and weights replicated per core.

Gather strategy (the hot loop): dma_gather (InstDMAGatherAnt) moves ~16
random 256B rows per DMA descriptor, but takes int16 indices (< 32768).
The feats table has 262144 rows, so each 512-voxel tile does a two-level
gather:
  L1: the tile's 13824 (offset, voxel) row-indices are bucketed by table
      chunk (8 chunks x 32768 rows) on the host; one dma_gather per chunk
      (2048-slot budget, dummy index 0 padding) pulls the rows chunk-local
      -> SBUF [128, 128, 128ch] bf16, then staged to a DRAM scratch region
      (16384 rows).
  L2: one dma_gather from the scratch (indices < 16384) restores
      (k, block, partition) order -> [128, 108, 128ch] bf16.
Per offset k: 4 PE transposes -> PSUM [128, 512], DVE copy -> bf16 rhs,
accumulating bf16 matmul W_k.T @ rhs -> PSUM [96, 512].
BN partial sum/sumsq via ACT accum; conv staged to DRAM channel-major.

Two NEFFs (the AllReduce-in-kernel path is unstable under the axon PJRT
bridge, so per-core BN partial sums are combined on the host -- 768 B of
float math -- between the two device passes):
  pass 2: y = relu(conv * scale + shift) (fused ACT op), PE transpose back
    to row-major, store. The host index permutation is chosen so pass2's
    output DMA writes contiguous 6KB runs per partition.
"""
import numpy as np
import ml_dtypes

import concourse.bass as bass
import concourse.bacc as bacc
import concourse.tile as tile
import concourse.mybir as mybir
from concourse.masks import make_identity

F32 = mybir.dt.float32
BF16 = mybir.dt.bfloat16
I16 = mybir.dt.int16
AF = mybir.ActivationFunctionType

N_TOTAL = 262144
C = 96
CP = 128                             # channel-padded row (256B bf16)
KVOL = 27
N_CORES = 8
N_PER_CORE = N_TOTAL // N_CORES      # 32768
TILE_V = 512                         # voxels per tile
BLOCKS = TILE_V // 128               # 4
J = KVOL * BLOCKS                    # 108 gathered row-blocks per tile
N_TILES = N_PER_CORE // TILE_V       # 64
NCHUNK = 8
CHUNK_ROWS = N_TOTAL // NCHUNK       # 32768 (int16-addressable)
SLOT_BUDGET = 2048                   # slots per (tile, chunk), mult of 128
SLOTS = NCHUNK * SLOT_BUDGET         # 16384 per tile
BN_EPS = 1e-5

_cache = {}


# --------------------------------------------------------------------------
# graph builders
# --------------------------------------------------------------------------
NI_MAX = 1024                        # hw limit: indices per dma_gather
NQUEUES = 4                          # SWDGE queues (ucode max 4)


def pass1_body(nc, feats_bf, idx1, idx2, Wt, convT, stats, n_tiles,
               tc=None, cpool=None):
    """Sparse-conv pass: gathers + matmuls + BN partial sums.

    When tc/cpool are given (fused single-NEFF build), runs inside the
    caller's TileContext and leaves stats in cpool tiles; `stats` may then
    be None. Standalone, creates its own context and writes `stats`."""
    scratch = nc.dram_tensor("scratch", [n_tiles, SLOTS, CP], BF16)
    nstripe = NI_MAX // 128          # stripes written per L1 gather
    l1_per_chunk = SLOT_BUDGET // NI_MAX
    qn = [0]

    def next_q():
        qn[0] = (qn[0] + 1) % NQUEUES
        return qn[0]

    import contextlib
    own_ctx = tc is None
    ctx = contextlib.ExitStack()
    with ctx:
        if own_ctx:
            tc = ctx.enter_context(tile.TileContext(nc))
            cpool = ctx.enter_context(tc.tile_pool(name="const", bufs=1))
        with (
            tc.tile_pool(name="gp", bufs=2) as gpool,
            tc.tile_pool(name="g2p", bufs=2) as g2pool,
            tc.tile_pool(name="rp", bufs=3) as rpool,
            tc.tile_pool(name="sp", bufs=2) as spool,
            tc.tile_pool(name="psA", bufs=2, space="PSUM") as psA,
            tc.tile_pool(name="psB", bufs=2, space="PSUM") as psB,
        ):
            ident = cpool.tile([128, 128], BF16, tag="ident")
            make_identity(nc, ident[:])
            w_sb = cpool.tile([CP, KVOL, C], BF16, tag="w")
            nc.sync.dma_start(w_sb[:], Wt[:])

            sum_acc = cpool.tile([C, n_tiles], F32, tag="sum_acc")
            sq_acc = cpool.tile([C, n_tiles], F32, tag="sq_acc")

            def emit_front(t):
                """idx loads + L1 gathers + scratch store for tile t."""
                i1 = gpool.tile([128, NCHUNK, SLOT_BUDGET // 16], I16,
                                tag="i1")
                nc.sync.dma_start(i1[:], idx1[t])
                i2 = gpool.tile([128, J * 128 // 16], I16, tag="i2")
                nc.sync.dma_start(i2[:], idx2[t])
                gs = gpool.tile([128, SLOTS // 128, CP], BF16, tag="gs")
                for c in range(NCHUNK):
                    for h in range(l1_per_chunk):
                        s0 = (c * l1_per_chunk + h) * nstripe
                        nc.gpsimd.dma_gather(
                            out_ap=gs[:, s0:s0 + nstripe, :],
                            in_ap=feats_bf[c * CHUNK_ROWS:
                                           (c + 1) * CHUNK_ROWS, :],
                            idxs_ap=i1[:, c, h * (NI_MAX // 16):
                                       (h + 1) * (NI_MAX // 16)],
                            num_idxs=NI_MAX,
                            num_idxs_reg=NI_MAX,
                            elem_size=CP,
                            queue_num=next_q(),
                        )
                # stage to DRAM: slot s -> scratch row (s%128)*128 + s//128
                nc.sync.dma_start(
                    scratch[t].rearrange("(p s) c -> p s c", p=128), gs[:])
                return i2

            def emit_back(t, i2):
                """L2 gather-back + matmuls + stats for tile t."""
                g2 = g2pool.tile([128, J, CP], BF16, tag="g2")
                for m in range((J * 128 + NI_MAX - 1) // NI_MAX):
                    ni = min(NI_MAX, J * 128 - m * NI_MAX)
                    nc.gpsimd.dma_gather(
                        out_ap=g2[:, m * nstripe:m * nstripe + ni // 128, :],
                        in_ap=scratch[t],
                        idxs_ap=i2[:, m * (NI_MAX // 16):
                                   m * (NI_MAX // 16) + ni // 16],
                        num_idxs=ni,
                        num_idxs_reg=ni,
                        elem_size=CP,
                        queue_num=next_q(),
                    )

                out_ps = psA.tile([C, TILE_V], F32, tag="outp")
                for k in range(KVOL):
                    tp = psB.tile([128, TILE_V], BF16, tag="tp")
                    for b in range(BLOCKS):
                        nc.tensor.transpose(
                            tp[:, b * 128:(b + 1) * 128],
                            g2[:, k * BLOCKS + b, :],
                            ident[:],
                        )
                    rhs = rpool.tile([128, TILE_V], BF16, tag="rhs")
                    nc.vector.tensor_copy(rhs[:], tp[:])
                    nc.tensor.matmul(
                        out_ps[:], w_sb[:, k, :], rhs[:],
                        start=(k == 0), stop=(k == KVOL - 1),
                    )

                conv_sb = spool.tile([C, TILE_V], F32, tag="conv")
                nc.scalar.activation(
                    conv_sb[:], out_ps[:], AF.Identity,
                    accum_out=sum_acc[:, t:t + 1])
                sq_sb = spool.tile([C, TILE_V], F32, tag="sq")
                nc.scalar.activation(
                    sq_sb[:], conv_sb[:], AF.Square,
                    accum_out=sq_acc[:, t:t + 1])
                nc.sync.dma_start(
                    convT[:, t * TILE_V:(t + 1) * TILE_V], conv_sb[:])

            # software pipeline: emit tile t+1's gathers before tile t's
            # gather-back so the gpsimd engine never stalls on the scratch
            # store of the current tile.
            pend = None
            for t in range(n_tiles):
                i2 = emit_front(t)
                if pend is not None:
                    emit_back(pend[0], pend[1])
                pend = (t, i2)
            emit_back(pend[0], pend[1])

            stats_sb = cpool.tile([C, 2], F32, tag="stats_sb")
            nc.vector.reduce_sum(
                stats_sb[:, 0:1], sum_acc[:], axis=mybir.AxisListType.X)
            nc.vector.reduce_sum(
                stats_sb[:, 1:2], sq_acc[:], axis=mybir.AxisListType.X)
            if stats is not None:
                nc.sync.dma_start(stats[:], stats_sb[:])
    return stats_sb


def pass2_body(nc, tc, cpool, convT, scale_sb, shift_sb, out, n_tiles):
    """Normalize + ReLU + transpose back to row-major."""
    tile_v2, n_tiles2 = _tile2(n_tiles)
    blocks2 = tile_v2 // 128
    with (
        tc.tile_pool(name="sp2", bufs=3) as spool,
        tc.tile_pool(name="ps2", bufs=4, space="PSUM") as ps,
    ):
        ident = cpool.tile([C, C], F32, tag="ident2")
        make_identity(nc, ident[:])
        for t in range(n_tiles2):
            nsb = spool.tile([C, tile_v2], F32, tag="nsb")
            nc.sync.dma_start(
                nsb[:], convT[:, t * tile_v2:(t + 1) * tile_v2])
            nrm = spool.tile([C, tile_v2], F32, tag="nrm")
            nc.scalar.activation(
                nrm[:], nsb[:], AF.Relu,
                bias=shift_sb[:], scale=scale_sb[:])
            # conv column b*128 + p holds voxel t*tile_v2 + p*blocks2 + b
            # (host vperm), so partition p's osb row is a contiguous run of
            # blocks2 output rows.
            osb = spool.tile([128, blocks2 * C], F32, tag="osb")
            for g in range(0, blocks2, 4):
                gw = min(4, blocks2 - g)
                op = ps.tile([128, 4 * C], F32, tag="op")
                for bi in range(gw):
                    nc.tensor.transpose(
                        op[:, bi * C:(bi + 1) * C],
                        nrm[:, (g + bi) * 128:(g + bi + 1) * 128],
                        ident[:],
                    )
                nc.vector.tensor_copy(
                    osb[:, g * C:(g + gw) * C], op[:, :gw * C])
            nc.sync.dma_start(
                out[t * tile_v2:(t + 1) * tile_v2, :].rearrange(
                    "(p b) c -> p b c", p=128),
                osb[:].rearrange("p (b c) -> p b c", b=blocks2),
            )


def build_fused(n_tiles=N_TILES, n_cores=N_CORES):
    """Single NEFF: conv + BN-stats AllReduce + normalize/ReLU."""
    nc = bacc.Bacc("TRN2", target_bir_lowering=False, debug=False,
                   num_devices=n_cores, num_swdge_queues=NQUEUES)
    feats_bf = nc.dram_tensor("feats_bf", [N_TOTAL, CP], BF16,
                              kind="ExternalInput")
    idx1 = nc.dram_tensor("idx1", [n_tiles, 128, NCHUNK, SLOT_BUDGET // 16],
                          I16, kind="ExternalInput")
    idx2 = nc.dram_tensor("idx2", [n_tiles, 128, J * 128 // 16], I16,
                          kind="ExternalInput")
    Wt = nc.dram_tensor("Wt", [CP, KVOL, C], BF16, kind="ExternalInput")
    gamma = nc.dram_tensor("gamma", [C, 1], F32, kind="ExternalInput")
    beta = nc.dram_tensor("beta", [C, 1], F32, kind="ExternalInput")
    out = nc.dram_tensor("out", [n_tiles * TILE_V, C], F32,
                         kind="ExternalOutput")
    convT = nc.dram_tensor("convT", [C, n_tiles * TILE_V], F32)
    bnsrc = nc.dram_tensor("bnsrc", [C, 2], F32)
    bnred = nc.dram_tensor("bnred", [C, 2], F32, addr_space="Shared")
    n_total = n_tiles * TILE_V * n_cores

    with tile.TileContext(nc) as tc:
        with tc.tile_pool(name="const", bufs=1) as cpool:
            stats_sb = pass1_body(nc, feats_bf, idx1, idx2, Wt, convT, None,
                                  n_tiles, tc=tc, cpool=cpool)
            # global BN stats: AllReduce the [C, 2] partial sums
            nc.sync.dma_start(bnsrc[:], stats_sb[:])
            nc.gpsimd.collective_compute(
                "AllReduce", mybir.AluOpType.add,
                replica_groups=[list(range(n_cores))],
                ins=[bnsrc[:]], outs=[bnred[:]],
            )
            st2 = cpool.tile([C, 2], F32, tag="st2")
            nc.sync.dma_start(st2[:], bnred[:])
            gam = cpool.tile([C, 1], F32, tag="gam")
            bet = cpool.tile([C, 1], F32, tag="bet")
            nc.sync.dma_start(gam[:], gamma[:])
            nc.sync.dma_start(bet[:], beta[:])

            # scale = gamma * rsqrt(var + eps); shift = beta - mean * scale
            mm = cpool.tile([C, 2], F32, tag="mm")
            nc.vector.tensor_scalar(
                mm[:], st2[:], 1.0 / n_total, None, mybir.AluOpType.mult)
            msq = cpool.tile([C, 1], F32, tag="msq")
            nc.vector.tensor_tensor(
                msq[:], mm[:, 0:1], mm[:, 0:1], mybir.AluOpType.mult)
            var = cpool.tile([C, 1], F32, tag="var")
            nc.vector.tensor_tensor(
                var[:], mm[:, 1:2], msq[:], mybir.AluOpType.subtract)
            epst = cpool.tile([C, 1], F32, tag="eps")
            nc.vector.memset(epst[:], BN_EPS)
            std = cpool.tile([C, 1], F32, tag="std")
            nc.scalar.activation(std[:], var[:], AF.Sqrt, bias=epst[:])
            inv = cpool.tile([C, 1], F32, tag="inv")
            nc.vector.reciprocal(inv[:], std[:])
            scale_sb = cpool.tile([C, 1], F32, tag="scale")
            nc.vector.tensor_tensor(
                scale_sb[:], gam[:], inv[:], mybir.AluOpType.mult)
            mscale = cpool.tile([C, 1], F32, tag="mscale")
            nc.vector.tensor_tensor(
                mscale[:], mm[:, 0:1], scale_sb[:], mybir.AluOpType.mult)
            shift_sb = cpool.tile([C, 1], F32, tag="shift")
            nc.vector.tensor_tensor(
                shift_sb[:], bet[:], mscale[:], mybir.AluOpType.subtract)

            pass2_body(nc, tc, cpool, convT, scale_sb, shift_sb, out, n_tiles)

    nc.finalize()
    return nc


def build_pass1(n_tiles=N_TILES, n_cores=N_CORES):
    nc = bacc.Bacc("TRN2", target_bir_lowering=False, debug=False,
                   num_devices=n_cores, num_swdge_queues=NQUEUES)
    feats_bf = nc.dram_tensor("feats_bf", [N_TOTAL, CP], BF16,
                              kind="ExternalInput")
    idx1 = nc.dram_tensor("idx1", [n_tiles, 128, NCHUNK, SLOT_BUDGET // 16],
                          I16, kind="ExternalInput")
    idx2 = nc.dram_tensor("idx2", [n_tiles, 128, J * 128 // 16], I16,
                          kind="ExternalInput")
    Wt = nc.dram_tensor("Wt", [CP, KVOL, C], BF16, kind="ExternalInput")
    convT = nc.dram_tensor("convT", [C, n_tiles * TILE_V], F32,
                           kind="ExternalOutput")
    stats = nc.dram_tensor("stats", [C, 2], F32, kind="ExternalOutput")
    pass1_body(nc, feats_bf, idx1, idx2, Wt, convT, stats, n_tiles)
    nc.finalize()
    return nc


def _tile2(n_tiles):
    """Pass-2 tiling: [tile_v2, n_tiles2]."""
    total = n_tiles * TILE_V
    tile_v2 = min(2048, total)
    return tile_v2, total // tile_v2


def build_pass2(n_tiles=N_TILES):
    nc = bacc.Bacc("TRN2", target_bir_lowering=False, debug=False,
                   num_devices=N_CORES)
    tile_v2, n_tiles2 = _tile2(n_tiles)
    blocks2 = tile_v2 // 128
    convT = nc.dram_tensor("convT", [C, n_tiles * TILE_V], F32,
                           kind="ExternalInput")
    scale = nc.dram_tensor("scale", [C, 1], F32, kind="ExternalInput")
    shift = nc.dram_tensor("shift", [C, 1], F32, kind="ExternalInput")
    out = nc.dram_tensor("out", [n_tiles * TILE_V, C], F32,
                         kind="ExternalOutput")

    with tile.TileContext(nc) as tc:
        with (
            tc.tile_pool(name="const", bufs=1) as cpool,
            tc.tile_pool(name="sp", bufs=3) as spool,
            tc.tile_pool(name="ps", bufs=4, space="PSUM") as ps,
        ):
            ident = cpool.tile([C, C], F32, tag="ident")
            make_identity(nc, ident[:])
            scale_sb = cpool.tile([C, 1], F32, tag="scale")
            shift_sb = cpool.tile([C, 1], F32, tag="shift")
            nc.sync.dma_start(scale_sb[:], scale[:])
            nc.sync.dma_start(shift_sb[:], shift[:])

            for t in range(n_tiles2):
                nsb = spool.tile([C, tile_v2], F32, tag="nsb")
                nc.sync.dma_start(
                    nsb[:], convT[:, t * tile_v2:(t + 1) * tile_v2])
                nrm = spool.tile([C, tile_v2], F32, tag="nrm")
                nc.scalar.activation(
                    nrm[:], nsb[:], AF.Relu,
                    bias=shift_sb[:], scale=scale_sb[:])
                # conv column b*128 + p holds voxel t*tile_v2 + p*blocks2 + b
                # (host vperm), so partition p's osb row is a contiguous run
                # of blocks2 output rows.
                osb = spool.tile([128, blocks2 * C], F32, tag="osb")
                for g in range(0, blocks2, 4):
                    gw = min(4, blocks2 - g)
                    op = ps.tile([128, 4 * C], F32, tag="op")
                    for bi in range(gw):
                        nc.tensor.transpose(
                            op[:, bi * C:(bi + 1) * C],
                            nrm[:, (g + bi) * 128:(g + bi + 1) * 128],
                            ident[:],
                        )
                    nc.vector.tensor_copy(
                        osb[:, g * C:(g + gw) * C], op[:, :gw * C])
                nc.sync.dma_start(
                    out[t * tile_v2:(t + 1) * tile_v2, :].rearrange(
                        "(p b) c -> p b c", p=128),
                    osb[:].rearrange("p (b c) -> p b c", b=blocks2),
                )

    nc.finalize()
    return nc


# --------------------------------------------------------------------------
# reusable PJRT runner (keeps the compiled executable across calls)
# --------------------------------------------------------------------------
class _Runner:
    """Runs a bass NEFF over n_cores devices via shard_map.

    `replicated`: input names fed once (same array on every core).
    Inputs/outputs are jax arrays; sharded inputs are globally concatenated
    on axis 0 (core-major). Outputs stay on device until converted.
    """

    def __init__(self, nc, n_cores, replicated=()):
        import jax
        from jax.sharding import Mesh, PartitionSpec
        from jax.experimental.shard_map import shard_map
        from concourse import bass2jax

        bass2jax.install_neuronx_cc_hook()
        self.jax = jax
        self.n_cores = n_cores
        self.replicated = set(replicated)
        pname = nc.partition_id_tensor.name if nc.partition_id_tensor else None
        in_names, out_names, out_avals, zero_outs = [], [], [], []
        for alloc in nc.m.functions[0].allocations:
            if not isinstance(alloc, mybir.MemoryLocationSet):
                continue
            name = alloc.memorylocations[0].name
            if alloc.kind == "ExternalInput":
                if name != pname:
                    in_names.append(name)
            elif alloc.kind == "ExternalOutput":
                out_names.append(name)
                shape = tuple(alloc.tensor_shape)
                dtype = mybir.dt.np(alloc.dtype)
                out_avals.append(jax.core.ShapedArray(shape, dtype))
                zero_outs.append(np.zeros(shape, dtype))
        self.in_names, self.out_names = in_names, out_names
        self.out_avals, self.zero_outs = out_avals, zero_outs
        n_params = len(in_names)
        self.n_params = n_params
        all_in = list(in_names) + list(out_names)
        if pname is not None:
            all_in.append(pname)

        def _body(*args):
            operands = list(args)
            if pname is not None:
                operands.append(bass2jax.partition_id_tensor())
            outs = bass2jax._bass_exec_p.bind(
                *operands,
                out_avals=tuple(out_avals),
                in_names=tuple(all_in),
                out_names=tuple(out_names),
                lowering_input_output_aliases=(),
                sim_require_finite=True,
                sim_require_nnan=True,
                nc=nc,
            )
            return tuple(outs)

        devices = jax.devices()[:n_cores]
        self.mesh = Mesh(np.asarray(devices), ("core",))
        self.in_specs = tuple(
            PartitionSpec() if n in self.replicated else PartitionSpec("core")
            for n in in_names
        ) + (PartitionSpec("core"),) * len(out_names)
        self.fn = jax.jit(
            shard_map(_body, mesh=self.mesh, in_specs=self.in_specs,
                      out_specs=(PartitionSpec("core"),) * len(out_names),
                      check_rep=False),
            keep_unused=True,
        )

    def prep(self, in_map):
        """in_map: replicated name -> array; sharded name -> list of per-core
        arrays OR pre-concatenated global array / jax array."""
        args = []
        for n in self.in_names:
            v = in_map[n]
            if isinstance(v, list):
                v = np.concatenate([np.asarray(x) for x in v], axis=0)
            args.append(v)
        args += [
            np.zeros((self.n_cores * z.shape[0], *z.shape[1:]), z.dtype)
            for z in self.zero_outs
        ]
        return args

    def prep_sharded(self, in_map):
        """Like prep, but device_put each arg with its target sharding so a
        subsequent fn(*args) does no data movement."""
        from jax.sharding import NamedSharding
        args = self.prep(in_map)
        out = []
        for a, spec in zip(args, self.in_specs):
            if not isinstance(a, self.jax.Array):
                a = self.jax.device_put(a, NamedSharding(self.mesh, spec))
            out.append(a)
        self.jax.block_until_ready(out)
        return out

    def run(self, in_map):
        outs = self.fn(*self.prep(in_map))
        self.jax.block_until_ready(outs)
        return dict(zip(self.out_names, outs))

    def percore(self, arr_global, name):
        i = self.out_names.index(name)
        return np.asarray(arr_global).reshape(
            self.n_cores, *self.out_avals[i].shape)


# --------------------------------------------------------------------------
# host-side glue
# --------------------------------------------------------------------------
def _vox_map(n_tiles):
    """vox[t, v]: shard-local voxel id gathered into conv column (t, v).

    Chosen so pass2's per-partition output rows are contiguous:
    for t = t2*G + q, v = b1*128 + p:
        vox = t2*tile_v2 + p*(4G) + q*4 + b1
    """
    tile_v2, n_t2 = _tile2(n_tiles)
    G = tile_v2 // TILE_V
    t2 = np.arange(n_t2)[:, None, None, None]
    q = np.arange(G)[None, :, None, None]
    b1 = np.arange(4)[None, None, :, None]
    p = np.arange(128)[None, None, None, :]
    vox = t2 * tile_v2 + p * (4 * G) + q * 4 + b1    # [n_t2, G, 4, 128]
    return vox.reshape(n_tiles, TILE_V)


def _wrap16(a):
    """[..., n] -> [..., 128, n//16]: position i -> (partition i%16, col
    i//16), replicated across the 8 groups of 16 partitions."""
    *lead, n = a.shape
    b = a.reshape(*lead, n // 16, 16)
    b = np.moveaxis(b, -1, -2)                       # [..., 16, n//16]
    return np.ascontiguousarray(
        np.broadcast_to(b[..., None, :, :],
                        (*lead, 8, 16, n // 16)).reshape(*lead, 128, n // 16))


def _arrange_idx(nbr_shard: np.ndarray, n_tiles: int):
    """Build (idx1, idx2) int16 arrays for one core shard.

    idx1 [n_tiles, 128, NCHUNK, SLOT_BUDGET//16]: L1 chunk-local row indices
        (slot-position order, dummy 0 padding).
    idx2 [n_tiles, 128, J*128//16]: L2 scratch-row indices restoring
        (k, block, partition) order.
    """
    vox = _vox_map(n_tiles)                          # [n_tiles, 512]
    R = nbr_shard[vox]                               # [n_tiles, 512, 27]
    R = np.ascontiguousarray(R.transpose(0, 2, 1)).reshape(
        n_tiles, KVOL * TILE_V)                      # i = k*512 + v
    chunk = (R >> 15).astype(np.int64)               # [n_tiles, 13824]
    local = (R & 32767).astype(np.int16)

    n = KVOL * TILE_V
    counts = np.zeros((n_tiles, NCHUNK), np.int64)
    trow = np.repeat(np.arange(n_tiles), n)
    np.add.at(counts, (trow, chunk.reshape(-1)), 1)
    if counts.max() > SLOT_BUDGET:
        raise RuntimeError(f"chunk bucket overflow: {counts.max()}")
    starts = np.concatenate(
        [np.zeros((n_tiles, 1), np.int64), np.cumsum(counts, 1)[:, :-1]], 1)

    # sort by full address (chunk major, row minor): ascending-address
    # gathers within each chunk bucket pipeline much better in HBM, and the
    # L2 unpermute absorbs any slot order.
    order = np.argsort(R, axis=1, kind="stable")
    sorted_chunk = np.take_along_axis(chunk, order, 1)
    within = np.arange(n)[None, :] - np.take_along_axis(
        starts, sorted_chunk, 1)
    rank = np.empty_like(within)
    np.put_along_axis(rank, order, within, 1)
    slot = chunk * SLOT_BUDGET + rank                # [n_tiles, 13824]

    l1 = np.zeros((n_tiles, SLOTS), np.int16)
    np.put_along_axis(l1, slot, local, 1)
    idx1 = _wrap16(l1.reshape(n_tiles, NCHUNK, SLOT_BUDGET))
    idx1 = np.ascontiguousarray(idx1.transpose(0, 2, 1, 3))

    dramrow = ((slot & 127) << 7) | (slot >> 7)      # (s%128)*128 + s//128
    idx2 = _wrap16(dramrow.astype(np.int16))
    return idx1, idx2


def _prep_pass1_inputs(feats, nbr, W, n_tiles):
    npc = n_tiles * TILE_V
    feats_bf = np.zeros((N_TOTAL, CP), ml_dtypes.bfloat16)
    feats_bf[:, :C] = feats.astype(ml_dtypes.bfloat16)
    Wt = np.zeros((CP, KVOL, C), ml_dtypes.bfloat16)
    Wt[:C] = W.transpose(1, 0, 2).astype(ml_dtypes.bfloat16)
    i1_all, i2_all = [], []
    for c in range(N_CORES):
        i1, i2 = _arrange_idx(nbr[c * npc:(c + 1) * npc], n_tiles)
        i1_all.append(i1)
        i2_all.append(i2)
    return {
        "feats_bf": feats_bf,
        "Wt": Wt,
        "idx1": np.concatenate(i1_all, 0),
        "idx2": np.concatenate(i2_all, 0),
    }


def _fused_in_map(feats, nbr, W, gamma, beta, n_tiles):
    in_map = _prep_pass1_inputs(feats, nbr, W, n_tiles)
    in_map["gamma"] = np.asarray(gamma, np.float32).reshape(C, 1)
    in_map["beta"] = np.asarray(beta, np.float32).reshape(C, 1)
    return in_map


def _fused_runner(n_tiles):
    key = ("fused", n_tiles)
    if key not in _cache:
        _cache[key] = _Runner(
            build_fused(n_tiles), N_CORES,
            replicated=("feats_bf", "Wt", "gamma", "beta"))
    return _cache[key]


def _two_pass_runners(n_tiles):
    key = ("p1", n_tiles)
    if key not in _cache:
        _cache[key] = _Runner(build_pass1(n_tiles), N_CORES,
                              replicated=("feats_bf", "Wt"))
    key2 = ("p2", n_tiles)
    if key2 not in _cache:
        _cache[key2] = _Runner(build_pass2(n_tiles), N_CORES,
                               replicated=("scale", "shift"))
    return _cache[key], _cache[key2]


def _host_bn(stats_percore, gamma, beta, n_total):
    """Combine per-core BN partial sums (768 B) into scale/shift."""
    s = stats_percore.sum(axis=0, dtype=np.float64)    # [96, 2]
    mean = s[:, 0] / n_total
    var = s[:, 1] / n_total - mean * mean
    inv = 1.0 / np.sqrt(var + BN_EPS)
    g = np.asarray(gamma, np.float64).reshape(C)
    b = np.asarray(beta, np.float64).reshape(C)
    scale = (g * inv).astype(np.float32)
    shift = (b - mean * g * inv).astype(np.float32)
    return scale.reshape(C, 1), shift.reshape(C, 1)


def run_pipeline(feats, nbr, W, gamma, beta, n_tiles):
    """Two NEFFs; the in-kernel AllReduce path (build_fused) measures ~6 ms
    slower with high variance under the axon bridge (software-emulated
    collectives), so BN stats are combined on the host instead."""
    r1, r2 = _two_pass_runners(n_tiles)
    res1 = r1.run(_prep_pass1_inputs(feats, nbr, W, n_tiles))
    stats = r1.percore(res1["stats"], "stats")         # [8, 96, 2]
    scale, shift = _host_bn(stats, gamma, beta, n_tiles * TILE_V * N_CORES)
    res2 = r2.run({"convT": res1["convT"], "scale": scale, "shift": shift})
    return np.asarray(res2["out"])


def kernel(feats, nbr_idx, W, gamma, beta):
    feats = np.ascontiguousarray(feats, dtype=np.float32)
    W = np.ascontiguousarray(W, dtype=np.float32)
    nbr = np.asarray(nbr_idx)
    gamma = np.asarray(gamma, dtype=np.float32)
    beta = np.asarray(beta, dtype=np.float32)
    return run_pipeline(feats, nbr, W, gamma, beta, N_TILES)


def _floor_runner():
    """Trivial 8-core kernel: measures the axon dispatch floor."""
    if "floor8" not in _cache:
        nc = bacc.Bacc("TRN2", target_bir_lowering=False, debug=False,
                       num_devices=N_CORES)
        x = nc.dram_tensor("x", [128, 128], F32, kind="ExternalInput")
        y = nc.dram_tensor("y", [128, 128], F32, kind="ExternalOutput")
        with tile.TileContext(nc) as tc:
            with tc.tile_pool(name="s", bufs=1) as p:
                t = p.tile([128, 128], F32, tag="t")
                nc.sync.dma_start(t[:], x[:])
                nc.vector.tensor_copy(t[:], t[:])
                nc.sync.dma_start(y[:], t[:])
        nc.finalize()
        _cache["floor8"] = _Runner(nc, N_CORES)
    return _cache["floor8"]


def _paired_diff(rf, args_f, rk, args_k, iters):
    """Median of adjacent (kernel - floor) wall-time differences. The axon
    dispatch overhead (~60-110 ms) drifts on a seconds scale; adjacent
    pairing cancels it. The 8 per-core NEFFs execute in parallel, so the
    difference is the per-core device time."""
    import time
    import jax

    jax.block_until_ready(rf.fn(*args_f))
    jax.block_until_ready(rk.fn(*args_k))
    diffs, floors, kerns = [], [], []
    for _ in range(iters):
        t0 = time.perf_counter()
        jax.block_until_ready(rf.fn(*args_f))
        tf = time.perf_counter() - t0
        t0 = time.perf_counter()
        jax.block_until_ready(rk.fn(*args_k))
        tk = time.perf_counter() - t0
        floors.append(tf)
        kerns.append(tk)
        diffs.append(tk - tf)
    return (float(np.median(diffs)), float(np.median(floors)),
            float(np.median(kerns)))


def measure_exec(feats, nbr_idx, W, gamma, beta, n_tiles=N_TILES, iters=14):
    """Paired-difference timing of both passes. Returns
    (pass1_s, pass2_s, floor_s)."""
    r1, r2 = _two_pass_runners(n_tiles)
    rf = _floor_runner()
    in1 = _prep_pass1_inputs(
        np.ascontiguousarray(feats, np.float32), np.asarray(nbr_idx),
        np.ascontiguousarray(W, np.float32), n_tiles)
    args1 = r1.prep_sharded(in1)
    args_f = rf.prep_sharded(
        {"x": np.ones((N_CORES * 128, 128), np.float32)})
    d1, f1, _ = _paired_diff(rf, args_f, r1, args1, iters)

    res1 = dict(zip(r1.out_names, r1.fn(*args1)))
    stats = r1.percore(np.asarray(res1["stats"]), "stats")
    scale, shift = _host_bn(stats, gamma, beta, n_tiles * TILE_V * N_CORES)
    args2 = r2.prep_sharded({
        "convT": res1["convT"], "scale": scale, "shift": shift})
    d2, f2, _ = _paired_diff(rf, args_f, r2, args2, iters)
    return d1, d2, (f1 + f2) / 2


# revision 22
# speedup vs baseline: 4414.2398x; 1.8196x over previous
"""Trainium2 Bass kernel for nn_BasicConvolutionBlock (sparse conv + BN + ReLU).

Math (per reference):
    conv[n] = sum_k feats[nbr_idx[n, k]] @ W[k]       # [N, 96], k = 0..26
    y = (conv - mean) * rsqrt(var + eps) * gamma + beta ; relu(y)

Distribution: voxel (N) dimension sharded across 8 NeuronCores; feats table
(bf16, channel-padded to 128) and weights replicated per core.

Gather strategy (the hot loop): dma_gather (InstDMAGatherAnt) moves ~16
random 256B rows per DMA descriptor, but takes int16 indices (< 32768).
The feats table has 262144 rows, so each 512-voxel tile does a two-level
gather:
  L1: the tile's 13824 (offset, voxel) row-indices are bucketed by table
      chunk (8 chunks x 32768 rows) on the host; one dma_gather per chunk
      (2048-slot budget, dummy index 0 padding) pulls the rows chunk-local
      -> SBUF [128, 128, 128ch] bf16, then staged to a DRAM scratch region
      (16384 rows).
  L2: one dma_gather from the scratch (indices < 16384) restores
      (k, block, partition) order -> [128, 108, 128ch] bf16.
Per offset k: 4 PE transposes -> PSUM [128, 512], DVE copy -> bf16 rhs,
accumulating bf16 matmul W_k.T @ rhs -> PSUM [96, 512].
BN partial sum/sumsq via ACT accum; conv staged to DRAM channel-major.

Two NEFFs (the AllReduce-in-kernel path is unstable under the axon PJRT
bridge, so per-core BN partial sums are combined on the host -- 768 B of
float math -- between the two device passes):
  pass 2: y = relu(conv * scale + shift) (fused ACT op), PE transpose back
    to row-major, store. The host index permutation is chosen so pass2's
    output DMA writes contiguous 6KB runs per partition.
"""
import numpy as np
import ml_dtypes

import concourse.bass as bass
import concourse.bacc as bacc
import concourse.tile as tile
import concourse.mybir as mybir
from concourse.masks import make_identity

F32 = mybir.dt.float32
BF16 = mybir.dt.bfloat16
I16 = mybir.dt.int16
AF = mybir.ActivationFunctionType

N_TOTAL = 262144
C = 96
CP = 128                             # channel-padded row (256B bf16)
KVOL = 27
N_CORES = 8
N_PER_CORE = N_TOTAL // N_CORES      # 32768
TILE_V = 512                         # voxels per tile
BLOCKS = TILE_V // 128               # 4
J = KVOL * BLOCKS                    # 108 gathered row-blocks per tile
N_TILES = N_PER_CORE // TILE_V       # 64
NCHUNK = 8
CHUNK_ROWS = N_TOTAL // NCHUNK       # 32768 (int16-addressable)
SLOT_BUDGET = 2048                   # slots per (tile, chunk), mult of 128
SLOTS = NCHUNK * SLOT_BUDGET         # 16384 per tile
BN_EPS = 1e-5

_cache = {}


# --------------------------------------------------------------------------
# graph builders
# --------------------------------------------------------------------------
NI_MAX = 1024                        # hw limit: indices per dma_gather
NQUEUES = 4                          # SWDGE queues (ucode max 4)


def pass1_body(nc, feats_bf, idx1, idx2, Wt, convT, stats, n_tiles,
               tc=None, cpool=None):
    """Sparse-conv pass: gathers + matmuls + BN partial sums.

    When tc/cpool are given (fused single-NEFF build), runs inside the
    caller's TileContext and leaves stats in cpool tiles; `stats` may then
    be None. Standalone, creates its own context and writes `stats`."""
    scratch = nc.dram_tensor("scratch", [n_tiles, SLOTS, CP], BF16)
    nstripe = NI_MAX // 128          # stripes written per L1 gather
    l1_per_chunk = SLOT_BUDGET // NI_MAX
    qn = [0]

    def next_q():
        qn[0] = (qn[0] + 1) % NQUEUES
        return qn[0]

    import contextlib
    own_ctx = tc is None
    ctx = contextlib.ExitStack()
    with ctx:
        if own_ctx:
            tc = ctx.enter_context(tile.TileContext(nc))
            cpool = ctx.enter_context(tc.tile_pool(name="const", bufs=1))
        with (
            tc.tile_pool(name="gp", bufs=2) as gpool,
            tc.tile_pool(name="g2p", bufs=2) as g2pool,
            tc.tile_pool(name="rp", bufs=3) as rpool,
            tc.tile_pool(name="sp", bufs=2) as spool,
            tc.tile_pool(name="psA", bufs=2, space="PSUM") as psA,
            tc.tile_pool(name="psB", bufs=2, space="PSUM") as psB,
        ):
            ident = cpool.tile([128, 128], BF16, tag="ident")
            make_identity(nc, ident[:])
            w_sb = cpool.tile([CP, KVOL, C], BF16, tag="w")
            nc.sync.dma_start(w_sb[:], Wt[:])

            sum_acc = cpool.tile([C, n_tiles], F32, tag="sum_acc")
            sq_acc = cpool.tile([C, n_tiles], F32, tag="sq_acc")

            def emit_front(t):
                """idx loads + L1 gathers + scratch store for tile t."""
                i1 = gpool.tile([128, NCHUNK, SLOT_BUDGET // 16], I16,
                                tag="i1")
                nc.sync.dma_start(i1[:], idx1[t])
                i2 = gpool.tile([128, J * 128 // 16], I16, tag="i2")
                nc.sync.dma_start(i2[:], idx2[t])
                gs = gpool.tile([128, SLOTS // 128, CP], BF16, tag="gs")
                for c in range(NCHUNK):
                    for h in range(l1_per_chunk):
                        s0 = (c * l1_per_chunk + h) * nstripe
                        nc.gpsimd.dma_gather(
                            out_ap=gs[:, s0:s0 + nstripe, :],
                            in_ap=feats_bf[c * CHUNK_ROWS:
                                           (c + 1) * CHUNK_ROWS, :],
                            idxs_ap=i1[:, c, h * (NI_MAX // 16):
                                       (h + 1) * (NI_MAX // 16)],
                            num_idxs=NI_MAX,
                            num_idxs_reg=NI_MAX,
                            elem_size=CP,
                            queue_num=next_q(),
                        )
                # stage to DRAM: slot s -> scratch row (s%128)*128 + s//128
                nc.sync.dma_start(
                    scratch[t].rearrange("(p s) c -> p s c", p=128), gs[:])
                return i2

            def emit_back(t, i2):
                """L2 gather-back + matmuls + stats for tile t."""
                g2 = g2pool.tile([128, J, CP], BF16, tag="g2")
                for m in range((J * 128 + NI_MAX - 1) // NI_MAX):
                    ni = min(NI_MAX, J * 128 - m * NI_MAX)
                    nc.gpsimd.dma_gather(
                        out_ap=g2[:, m * nstripe:m * nstripe + ni // 128, :],
                        in_ap=scratch[t],
                        idxs_ap=i2[:, m * (NI_MAX // 16):
                                   m * (NI_MAX // 16) + ni // 16],
                        num_idxs=ni,
                        num_idxs_reg=ni,
                        elem_size=CP,
                        queue_num=next_q(),
                    )

                out_ps = psA.tile([C, TILE_V], F32, tag="outp")
                for k in range(KVOL):
                    tp = psB.tile([128, TILE_V], BF16, tag="tp")
                    for b in range(BLOCKS):
                        nc.tensor.transpose(
                            tp[:, b * 128:(b + 1) * 128],
                            g2[:, k * BLOCKS + b, :],
                            ident[:],
                        )
                    rhs = rpool.tile([128, TILE_V], BF16, tag="rhs")
                    nc.vector.tensor_copy(rhs[:], tp[:])
                    nc.tensor.matmul(
                        out_ps[:], w_sb[:, k, :], rhs[:],
                        start=(k == 0), stop=(k == KVOL - 1),
                    )

                conv_sb = spool.tile([C, TILE_V], F32, tag="conv")
                nc.scalar.activation(
                    conv_sb[:], out_ps[:], AF.Identity,
                    accum_out=sum_acc[:, t:t + 1])
                sq_sb = spool.tile([C, TILE_V], F32, tag="sq")
                nc.scalar.activation(
                    sq_sb[:], conv_sb[:], AF.Square,
                    accum_out=sq_acc[:, t:t + 1])
                nc.sync.dma_start(
                    convT[:, t * TILE_V:(t + 1) * TILE_V], conv_sb[:])

            # software pipeline: emit tile t+1's gathers before tile t's
            # gather-back so the gpsimd engine never stalls on the scratch
            # store of the current tile.
            pend = None
            for t in range(n_tiles):
                i2 = emit_front(t)
                if pend is not None:
                    emit_back(pend[0], pend[1])
                pend = (t, i2)
            emit_back(pend[0], pend[1])

            stats_sb = cpool.tile([C, 2], F32, tag="stats_sb")
            nc.vector.reduce_sum(
                stats_sb[:, 0:1], sum_acc[:], axis=mybir.AxisListType.X)
            nc.vector.reduce_sum(
                stats_sb[:, 1:2], sq_acc[:], axis=mybir.AxisListType.X)
            if stats is not None:
                nc.sync.dma_start(stats[:], stats_sb[:])
    return stats_sb


def pass2_body(nc, tc, cpool, convT, scale_sb, shift_sb, out, n_tiles):
    """Normalize + ReLU + transpose back to row-major."""
    tile_v2, n_tiles2 = _tile2(n_tiles)
    blocks2 = tile_v2 // 128
    with (
        tc.tile_pool(name="sp2", bufs=3) as spool,
        tc.tile_pool(name="ps2", bufs=4, space="PSUM") as ps,
    ):
        ident = cpool.tile([C, C], F32, tag="ident2")
        make_identity(nc, ident[:])
        for t in range(n_tiles2):
            nsb = spool.tile([C, tile_v2], F32, tag="nsb")
            nc.sync.dma_start(
                nsb[:], convT[:, t * tile_v2:(t + 1) * tile_v2])
            nrm = spool.tile([C, tile_v2], F32, tag="nrm")
            nc.scalar.activation(
                nrm[:], nsb[:], AF.Relu,
                bias=shift_sb[:], scale=scale_sb[:])
            # conv column b*128 + p holds voxel t*tile_v2 + p*blocks2 + b
            # (host vperm), so partition p's osb row is a contiguous run of
            # blocks2 output rows.
            osb = spool.tile([128, blocks2 * C], F32, tag="osb")
            for g in range(0, blocks2, 4):
                gw = min(4, blocks2 - g)
                op = ps.tile([128, 4 * C], F32, tag="op")
                for bi in range(gw):
                    nc.tensor.transpose(
                        op[:, bi * C:(bi + 1) * C],
                        nrm[:, (g + bi) * 128:(g + bi + 1) * 128],
                        ident[:],
                    )
                nc.vector.tensor_copy(
                    osb[:, g * C:(g + gw) * C], op[:, :gw * C])
            nc.sync.dma_start(
                out[t * tile_v2:(t + 1) * tile_v2, :].rearrange(
                    "(p b) c -> p b c", p=128),
                osb[:].rearrange("p (b c) -> p b c", b=blocks2),
            )


def build_fused(n_tiles=N_TILES, n_cores=N_CORES):
    """Single NEFF: conv + BN-stats AllReduce + normalize/ReLU."""
    nc = bacc.Bacc("TRN2", target_bir_lowering=False, debug=False,
                   num_devices=n_cores, num_swdge_queues=NQUEUES)
    feats_bf = nc.dram_tensor("feats_bf", [N_TOTAL, CP], BF16,
                              kind="ExternalInput")
    idx1 = nc.dram_tensor("idx1", [n_tiles, 128, NCHUNK, SLOT_BUDGET // 16],
                          I16, kind="ExternalInput")
    idx2 = nc.dram_tensor("idx2", [n_tiles, 128, J * 128 // 16], I16,
                          kind="ExternalInput")
    Wt = nc.dram_tensor("Wt", [CP, KVOL, C], BF16, kind="ExternalInput")
    gamma = nc.dram_tensor("gamma", [C, 1], F32, kind="ExternalInput")
    beta = nc.dram_tensor("beta", [C, 1], F32, kind="ExternalInput")
    out = nc.dram_tensor("out", [n_tiles * TILE_V, C], F32,
                         kind="ExternalOutput")
    convT = nc.dram_tensor("convT", [C, n_tiles * TILE_V], F32)
    bnsrc = nc.dram_tensor("bnsrc", [C, 2], F32)
    bnred = nc.dram_tensor("bnred", [C, 2], F32, addr_space="Shared")
    n_total = n_tiles * TILE_V * n_cores

    with tile.TileContext(nc) as tc:
        with tc.tile_pool(name="const", bufs=1) as cpool:
            stats_sb = pass1_body(nc, feats_bf, idx1, idx2, Wt, convT, None,
                                  n_tiles, tc=tc, cpool=cpool)
            # global BN stats: AllReduce the [C, 2] partial sums
            nc.sync.dma_start(bnsrc[:], stats_sb[:])
            nc.gpsimd.collective_compute(
                "AllReduce", mybir.AluOpType.add,
                replica_groups=[list(range(n_cores))],
                ins=[bnsrc[:]], outs=[bnred[:]],
            )
            st2 = cpool.tile([C, 2], F32, tag="st2")
            nc.sync.dma_start(st2[:], bnred[:])
            gam = cpool.tile([C, 1], F32, tag="gam")
            bet = cpool.tile([C, 1], F32, tag="bet")
            nc.sync.dma_start(gam[:], gamma[:])
            nc.sync.dma_start(bet[:], beta[:])

            # scale = gamma * rsqrt(var + eps); shift = beta - mean * scale
            mm = cpool.tile([C, 2], F32, tag="mm")
            nc.vector.tensor_scalar(
                mm[:], st2[:], 1.0 / n_total, None, mybir.AluOpType.mult)
            msq = cpool.tile([C, 1], F32, tag="msq")
            nc.vector.tensor_tensor(
                msq[:], mm[:, 0:1], mm[:, 0:1], mybir.AluOpType.mult)
            var = cpool.tile([C, 1], F32, tag="var")
            nc.vector.tensor_tensor(
                var[:], mm[:, 1:2], msq[:], mybir.AluOpType.subtract)
            epst = cpool.tile([C, 1], F32, tag="eps")
            nc.vector.memset(epst[:], BN_EPS)
            std = cpool.tile([C, 1], F32, tag="std")
            nc.scalar.activation(std[:], var[:], AF.Sqrt, bias=epst[:])
            inv = cpool.tile([C, 1], F32, tag="inv")
            nc.vector.reciprocal(inv[:], std[:])
            scale_sb = cpool.tile([C, 1], F32, tag="scale")
            nc.vector.tensor_tensor(
                scale_sb[:], gam[:], inv[:], mybir.AluOpType.mult)
            mscale = cpool.tile([C, 1], F32, tag="mscale")
            nc.vector.tensor_tensor(
                mscale[:], mm[:, 0:1], scale_sb[:], mybir.AluOpType.mult)
            shift_sb = cpool.tile([C, 1], F32, tag="shift")
            nc.vector.tensor_tensor(
                shift_sb[:], bet[:], mscale[:], mybir.AluOpType.subtract)

            pass2_body(nc, tc, cpool, convT, scale_sb, shift_sb, out, n_tiles)

    nc.finalize()
    return nc


def build_pass1(n_tiles=N_TILES, n_cores=N_CORES):
    nc = bacc.Bacc("TRN2", target_bir_lowering=False, debug=False,
                   num_devices=n_cores, num_swdge_queues=NQUEUES)
    feats_bf = nc.dram_tensor("feats_bf", [N_TOTAL, CP], BF16,
                              kind="ExternalInput")
    idx1 = nc.dram_tensor("idx1", [n_tiles, 128, NCHUNK, SLOT_BUDGET // 16],
                          I16, kind="ExternalInput")
    idx2 = nc.dram_tensor("idx2", [n_tiles, 128, J * 128 // 16], I16,
                          kind="ExternalInput")
    Wt = nc.dram_tensor("Wt", [CP, KVOL, C], BF16, kind="ExternalInput")
    convT = nc.dram_tensor("convT", [C, n_tiles * TILE_V], F32,
                           kind="ExternalOutput")
    stats = nc.dram_tensor("stats", [C, 2], F32, kind="ExternalOutput")
    pass1_body(nc, feats_bf, idx1, idx2, Wt, convT, stats, n_tiles)
    nc.finalize()
    return nc


def _tile2(n_tiles):
    """Pass-2 tiling: [tile_v2, n_tiles2]."""
    total = n_tiles * TILE_V
    tile_v2 = min(2048, total)
    return tile_v2, total // tile_v2


def build_pass2(n_tiles=N_TILES):
    nc = bacc.Bacc("TRN2", target_bir_lowering=False, debug=False,
                   num_devices=N_CORES)
    tile_v2, n_tiles2 = _tile2(n_tiles)
    blocks2 = tile_v2 // 128
    convT = nc.dram_tensor("convT", [C, n_tiles * TILE_V], F32,
                           kind="ExternalInput")
    scale = nc.dram_tensor("scale", [C, 1], F32, kind="ExternalInput")
    shift = nc.dram_tensor("shift", [C, 1], F32, kind="ExternalInput")
    out = nc.dram_tensor("out", [n_tiles * TILE_V, C], F32,
                         kind="ExternalOutput")

    with tile.TileContext(nc) as tc:
        with (
            tc.tile_pool(name="const", bufs=1) as cpool,
            tc.tile_pool(name="sp", bufs=3) as spool,
            tc.tile_pool(name="ps", bufs=4, space="PSUM") as ps,
        ):
            ident = cpool.tile([C, C], F32, tag="ident")
            make_identity(nc, ident[:])
            scale_sb = cpool.tile([C, 1], F32, tag="scale")
            shift_sb = cpool.tile([C, 1], F32, tag="shift")
            nc.sync.dma_start(scale_sb[:], scale[:])
            nc.sync.dma_start(shift_sb[:], shift[:])

            for t in range(n_tiles2):
                nsb = spool.tile([C, tile_v2], F32, tag="nsb")
                nc.sync.dma_start(
                    nsb[:], convT[:, t * tile_v2:(t + 1) * tile_v2])
                nrm = spool.tile([C, tile_v2], F32, tag="nrm")
                nc.scalar.activation(
                    nrm[:], nsb[:], AF.Relu,
                    bias=shift_sb[:], scale=scale_sb[:])
                # conv column b*128 + p holds voxel t*tile_v2 + p*blocks2 + b
                # (host vperm), so partition p's osb row is a contiguous run
                # of blocks2 output rows.
                osb = spool.tile([128, blocks2 * C], F32, tag="osb")
                for g in range(0, blocks2, 4):
                    gw = min(4, blocks2 - g)
                    op = ps.tile([128, 4 * C], F32, tag="op")
                    for bi in range(gw):
                        nc.tensor.transpose(
                            op[:, bi * C:(bi + 1) * C],
                            nrm[:, (g + bi) * 128:(g + bi + 1) * 128],
                            ident[:],
                        )
                    nc.vector.tensor_copy(
                        osb[:, g * C:(g + gw) * C], op[:, :gw * C])
                nc.sync.dma_start(
                    out[t * tile_v2:(t + 1) * tile_v2, :].rearrange(
                        "(p b) c -> p b c", p=128),
                    osb[:].rearrange("p (b c) -> p b c", b=blocks2),
                )

    nc.finalize()
    return nc


# --------------------------------------------------------------------------
# reusable PJRT runner (keeps the compiled executable across calls)
# --------------------------------------------------------------------------
class _Runner:
    """Runs a bass NEFF over n_cores devices via shard_map.

    `replicated`: input names fed once (same array on every core).
    Inputs/outputs are jax arrays; sharded inputs are globally concatenated
    on axis 0 (core-major). Outputs stay on device until converted.
    """

    def __init__(self, nc, n_cores, replicated=()):
        import jax
        from jax.sharding import Mesh, PartitionSpec
        from jax.experimental.shard_map import shard_map
        from concourse import bass2jax

        bass2jax.install_neuronx_cc_hook()
        self.jax = jax
        self.n_cores = n_cores
        self.replicated = set(replicated)
        pname = nc.partition_id_tensor.name if nc.partition_id_tensor else None
        in_names, out_names, out_avals, zero_outs = [], [], [], []
        for alloc in nc.m.functions[0].allocations:
            if not isinstance(alloc, mybir.MemoryLocationSet):
                continue
            name = alloc.memorylocations[0].name
            if alloc.kind == "ExternalInput":
                if name != pname:
                    in_names.append(name)
            elif alloc.kind == "ExternalOutput":
                out_names.append(name)
                shape = tuple(alloc.tensor_shape)
                dtype = mybir.dt.np(alloc.dtype)
                out_avals.append(jax.core.ShapedArray(shape, dtype))
                zero_outs.append(np.zeros(shape, dtype))
        self.in_names, self.out_names = in_names, out_names
        self.out_avals, self.zero_outs = out_avals, zero_outs
        n_params = len(in_names)
        self.n_params = n_params
        all_in = list(in_names) + list(out_names)
        if pname is not None:
            all_in.append(pname)

        def _body(*args):
            operands = list(args)
            if pname is not None:
                operands.append(bass2jax.partition_id_tensor())
            outs = bass2jax._bass_exec_p.bind(
                *operands,
                out_avals=tuple(out_avals),
                in_names=tuple(all_in),
                out_names=tuple(out_names),
                lowering_input_output_aliases=(),
                sim_require_finite=True,
                sim_require_nnan=True,
                nc=nc,
            )
            return tuple(outs)

        devices = jax.devices()[:n_cores]
        self.mesh = Mesh(np.asarray(devices), ("core",))
        self.in_specs = tuple(
            PartitionSpec() if n in self.replicated else PartitionSpec("core")
            for n in in_names
        ) + (PartitionSpec("core"),) * len(out_names)
        self.fn = jax.jit(
            shard_map(_body, mesh=self.mesh, in_specs=self.in_specs,
                      out_specs=(PartitionSpec("core"),) * len(out_names),
                      check_rep=False),
            keep_unused=True,
        )

    def prep(self, in_map):
        """in_map: replicated name -> array; sharded name -> list of per-core
        arrays OR pre-concatenated global array / jax array."""
        args = []
        for n in self.in_names:
            v = in_map[n]
            if isinstance(v, list):
                v = np.concatenate([np.asarray(x) for x in v], axis=0)
            args.append(v)
        args += [
            np.zeros((self.n_cores * z.shape[0], *z.shape[1:]), z.dtype)
            for z in self.zero_outs
        ]
        return args

    def prep_sharded(self, in_map):
        """Like prep, but device_put each arg with its target sharding so a
        subsequent fn(*args) does no data movement."""
        from jax.sharding import NamedSharding
        args = self.prep(in_map)
        out = []
        for a, spec in zip(args, self.in_specs):
            if not isinstance(a, self.jax.Array):
                a = self.jax.device_put(a, NamedSharding(self.mesh, spec))
            out.append(a)
        self.jax.block_until_ready(out)
        return out

    def run(self, in_map):
        outs = self.fn(*self.prep(in_map))
        self.jax.block_until_ready(outs)
        return dict(zip(self.out_names, outs))

    def percore(self, arr_global, name):
        i = self.out_names.index(name)
        return np.asarray(arr_global).reshape(
            self.n_cores, *self.out_avals[i].shape)


# --------------------------------------------------------------------------
# host-side glue
# --------------------------------------------------------------------------
def _vox_map(n_tiles):
    """vox[t, v]: shard-local voxel id gathered into conv column (t, v).

    Chosen so pass2's per-partition output rows are contiguous:
    for t = t2*G + q, v = b1*128 + p:
        vox = t2*tile_v2 + p*(4G) + q*4 + b1
    """
    tile_v2, n_t2 = _tile2(n_tiles)
    G = tile_v2 // TILE_V
    t2 = np.arange(n_t2)[:, None, None, None]
    q = np.arange(G)[None, :, None, None]
    b1 = np.arange(4)[None, None, :, None]
    p = np.arange(128)[None, None, None, :]
    vox = t2 * tile_v2 + p * (4 * G) + q * 4 + b1    # [n_t2, G, 4, 128]
    return vox.reshape(n_tiles, TILE_V)


def _wrap16(a):
    """[..., n] -> [..., 128, n//16]: position i -> (partition i%16, col
    i//16), replicated across the 8 groups of 16 partitions."""
    *lead, n = a.shape
    b = a.reshape(*lead, n // 16, 16)
    b = np.moveaxis(b, -1, -2)                       # [..., 16, n//16]
    return np.ascontiguousarray(
        np.broadcast_to(b[..., None, :, :],
                        (*lead, 8, 16, n // 16)).reshape(*lead, 128, n // 16))


def _arrange_idx(nbr_shard: np.ndarray, n_tiles: int):
    """Build (idx1, idx2) int16 arrays for one core shard.

    idx1 [n_tiles, 128, NCHUNK, SLOT_BUDGET//16]: L1 chunk-local row indices
        (slot-position order, dummy 0 padding).
    idx2 [n_tiles, 128, J*128//16]: L2 scratch-row indices restoring
        (k, block, partition) order.
    """
    vox = _vox_map(n_tiles)                          # [n_tiles, 512]
    R = nbr_shard[vox]                               # [n_tiles, 512, 27]
    R = np.ascontiguousarray(R.transpose(0, 2, 1)).reshape(
        n_tiles, KVOL * TILE_V)                      # i = k*512 + v
    chunk = (R >> 15).astype(np.int64)               # [n_tiles, 13824]
    local = (R & 32767).astype(np.int16)

    n = KVOL * TILE_V
    counts = np.zeros((n_tiles, NCHUNK), np.int64)
    trow = np.repeat(np.arange(n_tiles), n)
    np.add.at(counts, (trow, chunk.reshape(-1)), 1)
    if counts.max() > SLOT_BUDGET:
        raise RuntimeError(f"chunk bucket overflow: {counts.max()}")
    starts = np.concatenate(
        [np.zeros((n_tiles, 1), np.int64), np.cumsum(counts, 1)[:, :-1]], 1)

    # sort by full address (chunk major, row minor): ascending-address
    # gathers within each chunk bucket pipeline much better in HBM, and the
    # L2 unpermute absorbs any slot order.
    order = np.argsort(R, axis=1, kind="stable")
    sorted_chunk = np.take_along_axis(chunk, order, 1)
    within = np.arange(n)[None, :] - np.take_along_axis(
        starts, sorted_chunk, 1)
    rank = np.empty_like(within)
    np.put_along_axis(rank, order, within, 1)
    slot = chunk * SLOT_BUDGET + rank                # [n_tiles, 13824]

    l1 = np.zeros((n_tiles, SLOTS), np.int16)
    np.put_along_axis(l1, slot, local, 1)
    idx1 = _wrap16(l1.reshape(n_tiles, NCHUNK, SLOT_BUDGET))
    idx1 = np.ascontiguousarray(idx1.transpose(0, 2, 1, 3))

    dramrow = ((slot & 127) << 7) | (slot >> 7)      # (s%128)*128 + s//128
    idx2 = _wrap16(dramrow.astype(np.int16))
    return idx1, idx2


def _prep_pass1_inputs(feats, nbr, W, n_tiles):
    npc = n_tiles * TILE_V
    feats_bf = np.zeros((N_TOTAL, CP), ml_dtypes.bfloat16)
    feats_bf[:, :C] = feats.astype(ml_dtypes.bfloat16)
    Wt = np.zeros((CP, KVOL, C), ml_dtypes.bfloat16)
    Wt[:C] = W.transpose(1, 0, 2).astype(ml_dtypes.bfloat16)
    i1_all, i2_all = [], []
    for c in range(N_CORES):
        i1, i2 = _arrange_idx(nbr[c * npc:(c + 1) * npc], n_tiles)
        i1_all.append(i1)
        i2_all.append(i2)
    return {
        "feats_bf": feats_bf,
        "Wt": Wt,
        "idx1": np.concatenate(i1_all, 0),
        "idx2": np.concatenate(i2_all, 0),
    }


def _fused_in_map(feats, nbr, W, gamma, beta, n_tiles):
    in_map = _prep_pass1_inputs(feats, nbr, W, n_tiles)
    in_map["gamma"] = np.asarray(gamma, np.float32).reshape(C, 1)
    in_map["beta"] = np.asarray(beta, np.float32).reshape(C, 1)
    return in_map


def _fused_runner(n_tiles):
    key = ("fused", n_tiles)
    if key not in _cache:
        _cache[key] = _Runner(
            build_fused(n_tiles), N_CORES,
            replicated=("feats_bf", "Wt", "gamma", "beta"))
    return _cache[key]


def _two_pass_runners(n_tiles):
    key = ("p1", n_tiles)
    if key not in _cache:
        _cache[key] = _Runner(build_pass1(n_tiles), N_CORES,
                              replicated=("feats_bf", "Wt"))
    key2 = ("p2", n_tiles)
    if key2 not in _cache:
        _cache[key2] = _Runner(build_pass2(n_tiles), N_CORES,
                               replicated=("scale", "shift"))
    return _cache[key], _cache[key2]


def _host_bn(stats_percore, gamma, beta, n_total):
    """Combine per-core BN partial sums (768 B) into scale/shift."""
    s = stats_percore.sum(axis=0, dtype=np.float64)    # [96, 2]
    mean = s[:, 0] / n_total
    var = s[:, 1] / n_total - mean * mean
    inv = 1.0 / np.sqrt(var + BN_EPS)
    g = np.asarray(gamma, np.float64).reshape(C)
    b = np.asarray(beta, np.float64).reshape(C)
    scale = (g * inv).astype(np.float32)
    shift = (b - mean * g * inv).astype(np.float32)
    return scale.reshape(C, 1), shift.reshape(C, 1)


def run_pipeline(feats, nbr, W, gamma, beta, n_tiles):
    """Two NEFFs; the in-kernel AllReduce path (build_fused) measures ~6 ms
    slower with high variance under the axon bridge (software-emulated
    collectives), so BN stats are combined on the host instead."""
    r1, r2 = _two_pass_runners(n_tiles)
    res1 = r1.run(_prep_pass1_inputs(feats, nbr, W, n_tiles))
    stats = r1.percore(res1["stats"], "stats")         # [8, 96, 2]
    scale, shift = _host_bn(stats, gamma, beta, n_tiles * TILE_V * N_CORES)
    res2 = r2.run({"convT": res1["convT"], "scale": scale, "shift": shift})
    return np.asarray(res2["out"])


def kernel(feats, nbr_idx, W, gamma, beta):
    feats = np.ascontiguousarray(feats, dtype=np.float32)
    W = np.ascontiguousarray(W, dtype=np.float32)
    nbr = np.asarray(nbr_idx)
    gamma = np.asarray(gamma, dtype=np.float32)
    beta = np.asarray(beta, dtype=np.float32)
    return run_pipeline(feats, nbr, W, gamma, beta, N_TILES)


def _floor_runner():
    """Trivial 8-core kernel: measures the axon dispatch floor."""
    if "floor8" not in _cache:
        nc = bacc.Bacc("TRN2", target_bir_lowering=False, debug=False,
                       num_devices=N_CORES)
        x = nc.dram_tensor("x", [128, 128], F32, kind="ExternalInput")
        y = nc.dram_tensor("y", [128, 128], F32, kind="ExternalOutput")
        with tile.TileContext(nc) as tc:
            with tc.tile_pool(name="s", bufs=1) as p:
                t = p.tile([128, 128], F32, tag="t")
                nc.sync.dma_start(t[:], x[:])
                nc.vector.tensor_copy(t[:], t[:])
                nc.sync.dma_start(y[:], t[:])
        nc.finalize()
        _cache["floor8"] = _Runner(nc, N_CORES)
    return _cache["floor8"]


def _paired_diff(rf, args_f, rk, args_k, iters):
    """Median of adjacent (kernel - floor) wall-time differences. The axon
    dispatch overhead (~60-110 ms) drifts on a seconds scale; adjacent
    pairing cancels it. The 8 per-core NEFFs execute in parallel, so the
    difference is the per-core device time."""
    import time
    import jax

    jax.block_until_ready(rf.fn(*args_f))
    jax.block_until_ready(rk.fn(*args_k))
    time.sleep(1.0)                  # let async transfers drain
    diffs, floors, kerns = [], [], []
    for i in range(iters):
        # alternate order within pairs so slow drift cancels to first order
        order = (rf, args_f, rk, args_k) if i % 2 == 0 else \
                (rk, args_k, rf, args_f)
        t0 = time.perf_counter()
        jax.block_until_ready(order[0].fn(*order[1]))
        ta = time.perf_counter() - t0
        t0 = time.perf_counter()
        jax.block_until_ready(order[2].fn(*order[3]))
        tb = time.perf_counter() - t0
        tf, tk = (ta, tb) if i % 2 == 0 else (tb, ta)
        floors.append(tf)
        kerns.append(tk)
        diffs.append(tk - tf)
    return (float(np.median(diffs)), float(np.median(floors)),
            float(np.median(kerns)))


def measure_exec(feats, nbr_idx, W, gamma, beta, n_tiles=N_TILES, iters=24):
    """Paired-difference timing of both passes. Returns
    (pass1_s, pass2_s, floor_s)."""
    r1, r2 = _two_pass_runners(n_tiles)
    rf = _floor_runner()
    in1 = _prep_pass1_inputs(
        np.ascontiguousarray(feats, np.float32), np.asarray(nbr_idx),
        np.ascontiguousarray(W, np.float32), n_tiles)
    args1 = r1.prep_sharded(in1)
    args_f = rf.prep_sharded(
        {"x": np.ones((N_CORES * 128, 128), np.float32)})
    d1, f1, _ = _paired_diff(rf, args_f, r1, args1, iters)

    res1 = dict(zip(r1.out_names, r1.fn(*args1)))
    stats = r1.percore(np.asarray(res1["stats"]), "stats")
    scale, shift = _host_bn(stats, gamma, beta, n_tiles * TILE_V * N_CORES)
    args2 = r2.prep_sharded({
        "convT": res1["convT"], "scale": scale, "shift": shift})
    d2, f2, _ = _paired_diff(rf, args_f, r2, args2, iters)
    return d1, d2, (f1 + f2) / 2
